# revision 6
# baseline (speedup 1.0000x reference)
"""Trainium2 Bass kernel: int4-quantized gate-proj (dequant matmul + qxscale + bias + silu).

Contract: kernel(**inputs) takes FULL unsharded numpy inputs (as produced by
setup_inputs) and returns the FULL [M, N] float32 output.

Sharding: column-parallel (Megatron gate_proj): the N=14336 output dim of
weight_i4 / weight_scale / bias is split into 8 shards of 1792; qx / qxscale
are replicated. Each NeuronCore computes out[:, shard] and the host
concatenates along axis 1.

Layout strategy (v2): the contraction dim K is consumed in a PERMUTED order
so that the packed int4 weights can be dequantized straight into matmul
layout with zero transposes on device:

  k-tile kt = 8*t + j  (t = 128-row block of packed words, j = nibble),
  partition p of tile kt holds original k = 1024*t + 8*p + j.

  - weights: host sends weight_i4.T as wpt[t, p, n] (p = packed-word index).
    On device, nibble j of wpt[t] is extracted with one DVE tensor_scalar
    (shift-left/arith-shift-right) giving the int4 for k = 1024t+8p+j at
    partition p -- already k-on-partitions. The group index for partition p
    is g = 8t + p//16 (constant per tile), so the host pre-expands
    weight_scale into wst[t, p, n] and a single DVE multiply produces the
    bf16 wT[k-tile] slab. wT (14.3 MB) stays resident in SBUF.
  - activations: host folds qxscale into qx, casts bf16, and pre-arranges
    xt[mb, p, kt, m] with the same k-permutation, blocked by 256-row
    m-blocks so each block is one contiguous 2.1 MB DMA.

Device loop is then a pure GEMM: for each 128-row m-tile, 32 k-tile
matmuls accumulate into 4 PSUM chunks (512/512/512/256 wide); eviction is
one DVE bias-add + one ACT native Silu per chunk, then a contiguous store.
"""

import os
import numpy as np
import ml_dtypes

import concourse.bass as bass
import concourse.mybir as mybir
import concourse.tile as tile
from concourse import bacc
from concourse._compat import with_exitstack
from concourse.bass_utils import run_bass_kernel_spmd

M, K, N, G = 4096, 4096, 14336, 128
NCORES = 8
NS = N // NCORES      # 1792 output columns per core
P = 128
KT = K // P           # 32 k-tiles
T = K // 8 // P       # 4 packed-word tiles (each yields 8 k-tiles)
MB = 256              # m-block rows per xt DMA
NMB = M // MB         # 16
CHUNKS = (512, 512, 512, 256)   # PSUM n-chunking of NS

f32 = mybir.dt.float32
bf16 = mybir.dt.bfloat16
i32 = mybir.dt.int32

BF16NP = ml_dtypes.bfloat16


@with_exitstack
def _emit(ctx, tc, xt, wpt, wst, bias, out):
    nc = tc.nc
    AL = mybir.AluOpType
    AF = mybir.ActivationFunctionType

    const = ctx.enter_context(tc.tile_pool(name="const", bufs=1))
    wprep = ctx.enter_context(tc.tile_pool(name="wprep", bufs=2))
    wres = ctx.enter_context(tc.tile_pool(name="wres", bufs=1))
    xload = ctx.enter_context(tc.tile_pool(name="xload", bufs=2))
    ev = ctx.enter_context(tc.tile_pool(name="ev", bufs=2))
    psum = ctx.enter_context(tc.tile_pool(name="psum", bufs=1, space="PSUM"))

    # prefetch first two x blocks ahead of the weight DMAs on the sync ring
    # so the first matmul isn't queued behind ~7 MB of W-prep transfers
    xq_pre = []
    for mb in range(2):
        xq = xload.tile([P, KT, MB], bf16, name="xq")
        nc.sync.dma_start(xq[:], xt[mb])
        xq_pre.append(xq)

    # ---- W prep: unpack + dequant straight into resident wT [p, kt, n] ----
    # GPSIMD (Pool) rejects tensor_scalar shift opcodes, so all dequant
    # runs on DVE
    GP_J = ()
    wT = wres.tile([P, KT, NS], bf16)
    bias_bc = None
    for t in range(T):
        wpt_sb = wprep.tile([P, NS], i32, name="wpt_sb")
        nc.sync.dma_start(wpt_sb[:], wpt[t])
        wst_sb = wprep.tile([P, NS], f32, name="wst_sb", bufs=1)
        nc.sync.dma_start(wst_sb[:], wst[t])
        if t == 0:
            # bias broadcast [P, NS]; needed at first eviction (~55us in)
            bias_bc = const.tile([P, NS], f32)
            nc.sync.dma_start(bias_bc[:], bias.to_broadcast((P, NS)))
        for j in range(8):
            kt = t * 8 + j
            eng = nc.gpsimd if j in GP_J else nc.vector
            nib = wprep.tile([P, NS], i32, bufs=1,
                             name="nib_g" if j in GP_J else "nib_v",
                             tag="nib_g" if j in GP_J else "nib_v")
            eng.tensor_scalar(
                out=nib[:], in0=wpt_sb[:],
                scalar1=28 - 4 * j, scalar2=28,
                op0=AL.logical_shift_left, op1=AL.arith_shift_right,
            )
            eng.tensor_tensor(
                out=wT[:, kt, :], in0=nib[:], in1=wst_sb[:], op=AL.mult,
            )

    # ---- main loop: pure GEMM over m ----
    for mb in range(NMB):
        if mb < 2:
            xq = xq_pre[mb]
        else:
            xq = xload.tile([P, KT, MB], bf16, name="xq")
            nc.sync.dma_start(xq[:], xt[mb])
        for mt2 in range(MB // P):
            m0 = mb * MB + mt2 * P
            psums = []
            off = 0
            for c, cw in enumerate(CHUNKS):
                psums.append((psum.tile([P, cw], f32, name=f"ps{c}",
                                        tag=f"ps{c}", bufs=2), off, cw))
                off += cw
            for kt in range(KT):
                lhsT = xq[:, kt, mt2 * P:(mt2 + 1) * P]
                for ps, off, cw in psums:
                    nc.tensor.matmul(
                        ps[:], lhsT, wT[:, kt, off:off + cw],
                        start=(kt == 0), stop=(kt == KT - 1),
                    )
            osb = ev.tile([P, NS], f32, name="osb")
            for ps, off, cw in psums:
                tmp = ev.tile([P, cw], f32, name="tmp", tag="tmp")
                nc.vector.tensor_tensor(out=tmp[:], in0=ps[:],
                                        in1=bias_bc[:, off:off + cw],
                                        op=AL.add)
                nc.scalar.activation(out=osb[:, off:off + cw], in_=tmp[:],
                                     func=AF.Silu)
                # store per chunk so the DMA of chunk c overlaps Silu of c+1
                nc.scalar.dma_start(out[m0:m0 + P, off:off + cw],
                                    osb[:, off:off + cw])


def build_nc():
    nc = bacc.Bacc("TRN2", target_bir_lowering=False, debug=False,
                   enable_asserts=False)
    xt = nc.dram_tensor("xt", [NMB, P, KT, MB], bf16, kind="ExternalInput").ap()
    wpt = nc.dram_tensor("wpt", [T, P, NS], i32, kind="ExternalInput").ap()
    wst = nc.dram_tensor("wst", [T, P, NS], f32, kind="ExternalInput").ap()
    bias = nc.dram_tensor("bias", [1, NS], f32, kind="ExternalInput").ap()
    out = nc.dram_tensor("out", [M, NS], f32, kind="ExternalOutput").ap()
    with tile.TileContext(nc) as tc:
        _emit(tc, xt, wpt, wst, bias, out)
    nc.compile()
    return nc


_NC_CACHE = {}


def _get_nc():
    if "nc" not in _NC_CACHE:
        _NC_CACHE["nc"] = build_nc()
    return _NC_CACHE["nc"]


def _prep_x(qx, qxscale):
    """Fold qxscale, cast bf16, and lay out xt[mb, p, kt, m] with the
    permuted k-order (k = 1024t + 8p + j, kt = 8t + j)."""
    xs = (qx * qxscale).astype(BF16NP)          # [M, K]
    # [M, K] -> (mb, mm, t, p, j) -> (mb, p, t, j, mm) -> [NMB, P, KT, MB]
    v = xs.reshape(NMB, MB, T, P, 8)
    return np.ascontiguousarray(v.transpose(0, 3, 2, 4, 1)).reshape(
        NMB, P, KT, MB)


def _make_in_maps(qx, qxscale, weight_i4, weight_scale, bias):
    xt = _prep_x(qx, qxscale)
    in_maps = []
    for c in range(NCORES):
        sl = slice(c * NS, (c + 1) * NS)
        wpt = np.ascontiguousarray(weight_i4[sl].T).reshape(T, P, NS)
        wst = np.ascontiguousarray(
            np.repeat(weight_scale[sl].T, 16, axis=0)).reshape(T, P, NS)
        in_maps.append({
            "xt": xt,
            "wpt": wpt,
            "wst": wst,
            "bias": np.ascontiguousarray(bias[sl]).reshape(1, NS),
        })
    return in_maps


def run(qx, qxscale, weight_i4, weight_scale, bias, trace=False, **spmd_kwargs):
    nc = _get_nc()
    in_maps = _make_in_maps(qx, qxscale, weight_i4, weight_scale, bias)
    res = run_bass_kernel_spmd(nc, in_maps, core_ids=list(range(NCORES)),
                               trace=trace, **spmd_kwargs)
    out = np.concatenate([res.results[c]["out"] for c in range(NCORES)],
                         axis=1)
    return out, res


def kernel(qx, qxscale, weight_i4, weight_scale, bias, group_size=G):
    gs = int(np.asarray(group_size))
    assert gs == G, f"kernel hardcodes group_size={G}, got {gs}"
    qx = np.ascontiguousarray(np.asarray(qx, dtype=np.float32))
    qxscale = np.ascontiguousarray(
        np.asarray(qxscale, dtype=np.float32).reshape(M, 1))
    weight_i4 = np.ascontiguousarray(np.asarray(weight_i4, dtype=np.int32))
    weight_scale = np.ascontiguousarray(
        np.asarray(weight_scale, dtype=np.float32))
    bias = np.ascontiguousarray(
        np.asarray(bias, dtype=np.float32).reshape(-1))
    out, _ = run(qx, qxscale, weight_i4, weight_scale, bias,
                 trace=bool(int(os.environ.get("GATEPROJ_TRACE", "0"))))
    return out


# revision 15
# speedup vs baseline: 1.0185x; 1.0185x over previous
"""Trainium2 Bass kernel: int4-quantized gate-proj (dequant matmul + qxscale + bias + silu).

Contract: kernel(**inputs) takes FULL unsharded numpy inputs (as produced by
setup_inputs) and returns the FULL [M, N] float32 output.

Sharding: column-parallel (Megatron gate_proj): the N=14336 output dim of
weight_i4 / weight_scale / bias is split into 8 shards of 1792; qx / qxscale
are replicated. Each NeuronCore computes out[:, shard] and the host
concatenates along axis 1.

Layout strategy (v2): the contraction dim K is consumed in a PERMUTED order
so that the packed int4 weights can be dequantized straight into matmul
layout with zero transposes on device:

  k-tile kt = 8*t + j  (t = 128-row block of packed words, j = nibble),
  partition p of tile kt holds original k = 1024*t + 8*p + j.

  - weights: host sends weight_i4.T as wpt[t, p, n] (p = packed-word index).
    On device, nibble j of wpt[t] is extracted with one DVE tensor_scalar
    (shift-left/arith-shift-right) giving the int4 for k = 1024t+8p+j at
    partition p -- already k-on-partitions. The group index for partition p
    is g = 8t + p//16 (constant per tile), so the host pre-expands
    weight_scale into wst[t, p, n] and a single DVE multiply produces the
    bf16 wT[k-tile] slab. wT (14.3 MB) stays resident in SBUF.
  - activations: host folds qxscale into qx, casts bf16, and pre-arranges
    xt[mb, p, kt, m] with the same k-permutation, blocked by 256-row
    m-blocks so each block is one contiguous 2.1 MB DMA.

Device loop is then a pure GEMM: for each 128-row m-tile, 32 k-tile
matmuls accumulate into 4 PSUM chunks (512/512/512/256 wide); eviction is
one DVE bias-add + one ACT native Silu per chunk, then a contiguous store.
"""

import os
import numpy as np
import ml_dtypes

import concourse.bass as bass
import concourse.mybir as mybir
import concourse.tile as tile
from concourse import bacc
from concourse._compat import with_exitstack
from concourse.bass_utils import run_bass_kernel_spmd

M, K, N, G = 4096, 4096, 14336, 128
NCORES = 8
NS = N // NCORES      # 1792 output columns per core
P = 128
KT = K // P           # 32 k-tiles
T = K // 8 // P       # 4 packed-word tiles (each yields 8 k-tiles)
MB = 256              # m-block rows per xt DMA
NMB = M // MB         # 16
CHUNKS = (512, 512, 512, 256)   # PSUM n-chunking of NS

f32 = mybir.dt.float32
bf16 = mybir.dt.bfloat16
i32 = mybir.dt.int32
i16 = mybir.dt.int16

BF16NP = ml_dtypes.bfloat16


@with_exitstack
def _emit(ctx, tc, xt, wpt, wst, bias, out):
    nc = tc.nc
    AL = mybir.AluOpType
    AF = mybir.ActivationFunctionType

    const = ctx.enter_context(tc.tile_pool(name="const", bufs=1))
    wprep = ctx.enter_context(tc.tile_pool(name="wprep", bufs=2))
    wres = ctx.enter_context(tc.tile_pool(name="wres", bufs=1))
    xload = ctx.enter_context(tc.tile_pool(name="xload", bufs=2))
    ev = ctx.enter_context(tc.tile_pool(name="ev", bufs=2))
    psum = ctx.enter_context(tc.tile_pool(name="psum", bufs=1, space="PSUM"))

    # prefetch x blocks on the SYNC ring; all W-prep DMAs go on the SCALAR
    # ring so the two HWDGE rings stream in parallel at startup
    xq_pre = []
    for mb in range(2):
        xq = xload.tile([P, KT, MB], bf16, name="xq")
        nc.sync.dma_start(xq[:], xt[mb])
        xq_pre.append(xq)

    # ---- W prep: unpack + dequant straight into per-kt resident tiles ----
    # DVE bit-vector ops are i32-only and Pool (gpsimd) rejects both the
    # shifts and mixed-dtype multiplies, so all dequant runs on DVE in i32.
    # Each k-tile gets its OWN tile so dequant writes never carry false WAR
    # deps against PE reads of earlier k-tiles (a single [P, KT, NS] tile
    # serialized DVE against PE in 1.5us lockstep steps).
    wT = [wres.tile([P, NS], bf16, name=f"wT{kt}", tag=f"wT{kt}")
          for kt in range(KT)]
    bias_bc = None
    for t in range(T):
        wpt_sb = wprep.tile([P, NS], i32, name="wpt_sb")
        nc.scalar.dma_start(wpt_sb[:], wpt[t])
        wst_sb = wprep.tile([P, NS], f32, name="wst_sb", bufs=1)
        nc.scalar.dma_start(wst_sb[:], wst[t])
        if t == 0:
            # bias broadcast [P, NS]; needed at first eviction (~55us in)
            bias_bc = const.tile([P, NS], f32)
            nc.scalar.dma_start(bias_bc[:], bias.to_broadcast((P, NS)))
        for j in range(8):
            kt = t * 8 + j
            nib = wprep.tile([P, NS], i32, name="nib")
            nc.vector.tensor_scalar(
                out=nib[:], in0=wpt_sb[:],
                scalar1=28 - 4 * j, scalar2=28,
                op0=AL.logical_shift_left, op1=AL.arith_shift_right,
            )
            nc.vector.tensor_tensor(
                out=wT[kt][:], in0=nib[:], in1=wst_sb[:], op=AL.mult,
            )

    # ---- main loop: pure GEMM over m ----
    for mb in range(NMB):
        if mb < 2:
            xq = xq_pre[mb]
        else:
            xq = xload.tile([P, KT, MB], bf16, name="xq")
            nc.sync.dma_start(xq[:], xt[mb])
        for mt2 in range(MB // P):
            m0 = mb * MB + mt2 * P
            psums = []
            off = 0
            for c, cw in enumerate(CHUNKS):
                psums.append((psum.tile([P, cw], f32, name=f"ps{c}",
                                        tag=f"ps{c}", bufs=2), off, cw))
                off += cw
            for kt in range(KT):
                lhsT = xq[:, kt, mt2 * P:(mt2 + 1) * P]
                for ps, off, cw in psums:
                    nc.tensor.matmul(
                        ps[:], lhsT, wT[kt][:, off:off + cw],
                        start=(kt == 0), stop=(kt == KT - 1),
                    )
            osb = ev.tile([P, NS], f32, name="osb")
            for ps, off, cw in psums:
                tmp = ev.tile([P, cw], f32, name="tmp", tag="tmp")
                nc.vector.tensor_tensor(out=tmp[:], in0=ps[:],
                                        in1=bias_bc[:, off:off + cw],
                                        op=AL.add)
                nc.scalar.activation(out=osb[:, off:off + cw], in_=tmp[:],
                                     func=AF.Silu)
                # store per chunk so the DMA of chunk c overlaps Silu of c+1
                nc.scalar.dma_start(out[m0:m0 + P, off:off + cw],
                                    osb[:, off:off + cw])


def build_nc():
    nc = bacc.Bacc("TRN2", target_bir_lowering=False, debug=False,
                   enable_asserts=False)
    xt = nc.dram_tensor("xt", [NMB, P, KT, MB], bf16, kind="ExternalInput").ap()
    wpt = nc.dram_tensor("wpt", [T, P, NS], i32, kind="ExternalInput").ap()
    wst = nc.dram_tensor("wst", [T, P, NS], f32, kind="ExternalInput").ap()
    bias = nc.dram_tensor("bias", [1, NS], f32, kind="ExternalInput").ap()
    out = nc.dram_tensor("out", [M, NS], f32, kind="ExternalOutput").ap()
    with tile.TileContext(nc) as tc:
        _emit(tc, xt, wpt, wst, bias, out)
    nc.compile()
    return nc


_NC_CACHE = {}


def _get_nc():
    if "nc" not in _NC_CACHE:
        _NC_CACHE["nc"] = build_nc()
    return _NC_CACHE["nc"]


def _prep_x(qx, qxscale):
    """Fold qxscale, cast bf16, and lay out xt[mb, p, kt, m] with the
    permuted k-order (k = 1024t + 8p + j, kt = 8t + j)."""
    xs = (qx * qxscale).astype(BF16NP)          # [M, K]
    # [M, K] -> (mb, mm, t, p, j) -> (mb, p, t, j, mm) -> [NMB, P, KT, MB]
    v = xs.reshape(NMB, MB, T, P, 8)
    return np.ascontiguousarray(v.transpose(0, 3, 2, 4, 1)).reshape(
        NMB, P, KT, MB)


def _make_in_maps(qx, qxscale, weight_i4, weight_scale, bias):
    xt = _prep_x(qx, qxscale)
    in_maps = []
    for c in range(NCORES):
        sl = slice(c * NS, (c + 1) * NS)
        wpt = np.ascontiguousarray(weight_i4[sl].T).reshape(T, P, NS)
        wst = np.ascontiguousarray(
            np.repeat(weight_scale[sl].T, 16, axis=0)).reshape(T, P, NS)
        in_maps.append({
            "xt": xt,
            "wpt": wpt,
            "wst": wst,
            "bias": np.ascontiguousarray(bias[sl]).reshape(1, NS),
        })
    return in_maps


def run(qx, qxscale, weight_i4, weight_scale, bias, trace=False, **spmd_kwargs):
    nc = _get_nc()
    in_maps = _make_in_maps(qx, qxscale, weight_i4, weight_scale, bias)
    res = run_bass_kernel_spmd(nc, in_maps, core_ids=list(range(NCORES)),
                               trace=trace, **spmd_kwargs)
    out = np.concatenate([res.results[c]["out"] for c in range(NCORES)],
                         axis=1)
    return out, res


def kernel(qx, qxscale, weight_i4, weight_scale, bias, group_size=G):
    gs = int(np.asarray(group_size))
    assert gs == G, f"kernel hardcodes group_size={G}, got {gs}"
    qx = np.ascontiguousarray(np.asarray(qx, dtype=np.float32))
    qxscale = np.ascontiguousarray(
        np.asarray(qxscale, dtype=np.float32).reshape(M, 1))
    weight_i4 = np.ascontiguousarray(np.asarray(weight_i4, dtype=np.int32))
    weight_scale = np.ascontiguousarray(
        np.asarray(weight_scale, dtype=np.float32))
    bias = np.ascontiguousarray(
        np.asarray(bias, dtype=np.float32).reshape(-1))
    out, _ = run(qx, qxscale, weight_i4, weight_scale, bias,
                 trace=bool(int(os.environ.get("GATEPROJ_TRACE", "0"))))
    return out
